# revision 7
# baseline (speedup 1.0000x reference)
"""4-layer GCN encoder on 8 Trainium2 NeuronCores.

Strategy (graph/data parallel, dst-node sharding):
  - Nodes are permuted into 8*NB blocks of 128 (balanced by in-degree) and
    sharded across 8 cores by destination.
  - Layer 1 dense projection x@W1 is computed fully on every core (cheaper
    than an AllGather of the result); layers 2-4 compute only the local node
    shard and AllGather the projected features.
  - Aggregation: per-edge gather of source rows via the SWDGE dma_gather
    instruction (int16 indices -> source-half split), then scatter-add via
    TensorEngine matmuls against on-chip-generated one-hot matrices S with
    the per-edge GCN norm as values (PSUM accumulation per dst block).
  - All matmul operands bf16, accumulation fp32.
"""

import math
import numpy as np
import ml_dtypes

import concourse.bacc as bacc
import concourse.mybir as mybir
import concourse.tile as tile
from concourse.bass_utils import run_bass_kernel_spmd

P = 128
BF16 = mybir.dt.bfloat16
F32 = mybir.dt.float32
I16 = mybir.dt.int16


class Cfg:
    def __init__(self, n_nodes=50000, n_edges=800000, in_ch=512, hid=256,
                 ncores=8, nb=49, G=7):
        self.n_nodes = n_nodes
        self.n_edges = n_edges
        self.in_ch = in_ch
        self.hid = hid
        self.ncores = ncores
        self.nb = nb                      # dst blocks of 128 per core
        self.G = G                        # blocks per gather group
        assert nb % G == 0
        self.NG = nb // G                 # groups per core
        self.shard = nb * P               # nodes per core (padded)
        self.npad = ncores * self.shard   # padded total nodes
        assert self.npad >= n_nodes
        self.half = self.npad // 2        # src-half boundary for int16 idx
        assert self.half % P == 0 and self.half < 32768
        self.fc_in = in_ch // P           # K chunks for layer 1
        self.fh = hid // P                # feature halves (2)
        assert self.fh == 2
        self.slab = 8                     # n-tiles per L1 x slab


CFG = Cfg()


# ----------------------------------------------------------------- host prep

def _preprocess(cfg, edge_index, edge_weight):
    """Numpy preprocessing: norms, balanced node permutation, per-core
    padded edge structures. Returns dict."""
    N = cfg.n_nodes
    src0 = np.asarray(edge_index[0], dtype=np.int64)
    dst0 = np.asarray(edge_index[1], dtype=np.int64)
    ew0 = np.asarray(edge_weight, dtype=np.float32)
    # self loops (PyG gcn_norm, fill=1)
    loops = np.arange(N, dtype=np.int64)
    src = np.concatenate([src0, loops])
    dst = np.concatenate([dst0, loops])
    ew = np.concatenate([ew0, np.ones(N, np.float32)])
    deg = np.bincount(dst, weights=ew.astype(np.float64), minlength=N)
    deg = deg.astype(np.float32)
    dis = np.where(deg > 0, 1.0 / np.sqrt(np.where(deg > 0, deg, 1.0)), 0.0)
    dis = dis.astype(np.float32)
    norm = dis[src] * ew * dis[dst]

    # balanced block assignment: round-robin of degree-sorted nodes
    NBT = cfg.ncores * cfg.nb
    degc = np.bincount(dst, minlength=N)          # in-edge counts per node
    order = np.argsort(-degc, kind="stable")
    blk_of_rank = np.arange(N) % NBT
    pos_of_rank = np.arange(N) // NBT
    assert pos_of_rank.max() < P, "block capacity exceeded"
    gslot = np.empty(N, dtype=np.int64)
    gslot[order] = blk_of_rank * P + pos_of_rank

    ps = gslot[src]
    pd = gslot[dst]

    # per (core, block, half) edge counts to find uniform tile count T
    eb = pd // P                                  # global dst block per edge
    ehalf = (ps >= cfg.half).astype(np.int64)
    key = eb * 2 + ehalf
    cnt = np.bincount(key, minlength=NBT * 2)
    T = max(1, int(np.ceil(cnt.max() / P)))

    nslots = cfg.nb * 2 * T                       # tiles per core
    cap = nslots * P
    gidx16 = np.zeros((cfg.ncores, 16, cap // 16), dtype=np.int16)
    dstc = np.zeros((cfg.ncores, P, nslots), dtype=np.float32)
    normc = np.zeros((cfg.ncores, P, nslots), dtype=np.float32)

    # global ordering of edges: core -> (group, half, block-in-group, tile)
    core_e = eb // cfg.nb
    b_in_core = eb % cfg.nb
    g = b_in_core // cfg.G
    bg = b_in_core % cfg.G
    # slot (tile) base for each edge's (b, h) bucket
    srt = np.lexsort((ps, ehalf, eb))             # sort edges by (block, half, src)
    # rank within bucket
    key_s = key[srt]
    # compute rank-in-bucket via cumcount
    uniq, inv, counts = np.unique(key_s, return_inverse=True, return_counts=True)
    starts = np.zeros_like(counts)
    starts[1:] = np.cumsum(counts)[:-1]
    rank_in_bucket = np.arange(len(srt)) - starts[inv]

    es = srt                                      # edge order
    t_idx = rank_in_bucket // P                   # tile within bucket
    j_idx = rank_in_bucket % P                    # lane within tile
    assert t_idx.max() < T
    sg = g[es]
    sh = ehalf[es]
    sbg = bg[es]
    s_slot = ((sg * 2 + sh) * cfg.G + sbg) * T + t_idx
    q = s_slot * P + j_idx                        # position within core arrays
    score = core_e[es]
    idxval = np.where(sh == 1, ps[es] - cfg.half, ps[es]).astype(np.int16)
    dlocal = (pd[es] % P).astype(np.float32)
    nval = norm[es]

    for c in range(cfg.ncores):
        m = score == c
        qc = q[m]
        gidx16[c, qc % 16, qc // 16] = idxval[m]
        dstc[c, qc % P, qc // P] = dlocal[m]
        normc[c, qc % P, qc // P] = nval[m]

    gidx = np.tile(gidx16, (1, 8, 1))             # replicate to 128 partitions
    inv_gslot = gslot                             # y[v] = yperm[gslot[v]]
    return dict(T=T, nslots=nslots, gidx=gidx,
                dstc=dstc.astype(ml_dtypes.bfloat16),
                normc=normc.astype(ml_dtypes.bfloat16),
                gslot=inv_gslot)


def _pack_xts(cfg, x, gslot):
    """Host: permuted, transposed, slab-tiled x for layer-1 lhsT streaming.
    Layout [fc, s, p, t*128+c] = x_perm[(s*8+t)*128+c, fc*128+p]."""
    xpad = np.zeros((cfg.npad, cfg.in_ch), dtype=np.float32)
    xpad[gslot] = x
    nslab = cfg.npad // (cfg.slab * P)
    a = xpad.T.reshape(cfg.fc_in, P, nslab, cfg.slab, P)
    a = a.transpose(0, 2, 1, 3, 4).reshape(cfg.fc_in, nslab, P, cfg.slab * P)
    return np.ascontiguousarray(a.astype(ml_dtypes.bfloat16)).reshape(
        cfg.fc_in * nslab * P, cfg.slab * P)


def _pack_wcat(cfg, Ws):
    """[128, (fc_in + 3*fh)*hid] bf16 : W1 chunks then W2..W4 chunks."""
    cols = []
    for Wl in Ws:
        k = Wl.shape[0]
        for fc in range(k // P):
            cols.append(Wl[fc * P:(fc + 1) * P, :])
    return np.concatenate(cols, axis=1).astype(ml_dtypes.bfloat16)


def _pack_bias(cfg, bs):
    out = np.zeros((P, 2 * len(bs)), dtype=np.float32)
    for l, b in enumerate(bs):
        for fh in range(cfg.fh):
            out[:, l * 2 + fh] = b[fh * P:(fh + 1) * P]
    return out


def _iota_np():
    return np.tile(np.arange(P, dtype=np.float32)[None, :], (P, 1)).astype(
        ml_dtypes.bfloat16)


# ----------------------------------------------------------------- builder

def _build(cfg, T, n_layers=4, debug_dense=False):
    nslots = cfg.nb * 2 * T
    HID = cfg.hid
    nc = bacc.Bacc("TRN2", target_bir_lowering=False, debug=False,
                   num_devices=cfg.ncores)

    gidx_d = nc.dram_tensor("gidx", [P, nslots * 8], I16, kind="ExternalInput")
    dstc_d = nc.dram_tensor("dstc", [P, nslots], BF16, kind="ExternalInput")
    normc_d = nc.dram_tensor("normc", [P, nslots], BF16, kind="ExternalInput")
    iota_d = nc.dram_tensor("iota", [P, P], BF16, kind="ExternalInput")
    wcat_cols = (cfg.fc_in + 3 * cfg.fh) * HID
    wcat_d = nc.dram_tensor("wcat", [P, wcat_cols], BF16, kind="ExternalInput")
    bias_d = nc.dram_tensor("bias", [P, 8], F32, kind="ExternalInput")
    prelu_d = nc.dram_tensor("prelua", [P, 2], F32, kind="ExternalInput")
    nslab = cfg.npad // (cfg.slab * P)
    xts_d = nc.dram_tensor("xts", [cfg.fc_in * nslab * P, cfg.slab * P], BF16,
                           kind="ExternalInput")
    out_d = nc.dram_tensor("out", [cfg.fh * cfg.nb * P, P], F32,
                           kind="ExternalOutput")

    w_off = {}
    off = 0
    for l in range(4):
        k = cfg.fc_in if l == 0 else cfg.fh
        for fc in range(k):
            w_off[(l, fc)] = off
            off += HID

    with tile.TileContext(nc) as tc:
        with (
            tc.tile_pool(name="res", bufs=1) as res,
            tc.tile_pool(name="mpool", bufs=2) as mpool,
            tc.tile_pool(name="spool", bufs=2) as spool,
            tc.tile_pool(name="xpool", bufs=2) as xpool,
            tc.tile_pool(name="apool", bufs=4) as apool,
            tc.tile_pool(name="htpool", bufs=1) as htpool,
            tc.tile_pool(name="opool", bufs=4) as opool,
            tc.tile_pool(name="ppool", bufs=cfg.G, space="PSUM") as ppool,
            tc.tile_pool(name="dpsum", bufs=1, space="PSUM") as dpsum,
            tc.tile_pool(name="dram", bufs=2, space="DRAM") as dram,
        ):
            # ---- resident loads
            gidx = res.tile([P, nslots * 8], I16)
            nc.sync.dma_start(out=gidx[:], in_=gidx_d[:])
            dstc = res.tile([P, nslots], BF16)
            nc.sync.dma_start(out=dstc[:], in_=dstc_d[:])
            normc = res.tile([P, nslots], BF16)
            nc.sync.dma_start(out=normc[:], in_=normc_d[:])
            iota = res.tile([P, P], BF16)
            nc.sync.dma_start(out=iota[:], in_=iota_d[:])
            wcat = res.tile([P, wcat_cols], BF16)
            nc.sync.dma_start(out=wcat[:], in_=wcat_d[:])
            bias = res.tile([P, 8], F32)
            nc.sync.dma_start(out=bias[:], in_=bias_d[:])
            prelua = res.tile([P, 2], F32)
            nc.sync.dma_start(out=prelua[:], in_=prelu_d[:])

            hT = {}

            def dense_full_l1(a_full):
                for s in range(nslab):
                    xsl = [xpool.tile([P, cfg.slab * P], BF16, tag=f"x{fc}", name=f"xsl{fc}")
                           for fc in range(cfg.fc_in)]
                    for fc in range(cfg.fc_in):
                        base = (fc * nslab + s) * P
                        nc.sync.dma_start(out=xsl[fc][:],
                                          in_=xts_d[base:base + P, :])
                    for t in range(cfg.slab):
                        nt = s * cfg.slab + t
                        pd_ = dpsum.tile([P, HID], F32, tag="dps", name="pd1")
                        for fc in range(cfg.fc_in):
                            nc.tensor.matmul(
                                out=pd_[:],
                                lhsT=xsl[fc][:, t * P:(t + 1) * P],
                                rhs=wcat[:, w_off[(0, fc)]:w_off[(0, fc)] + HID],
                                start=(fc == 0), stop=(fc == cfg.fc_in - 1))
                        asb = apool.tile([P, HID], BF16, tag="asb", name="asb1")
                        nc.scalar.copy(out=asb[:], in_=pd_[:])
                        nc.sync.dma_start(
                            out=a_full[nt * P:(nt + 1) * P, :], in_=asb[:])

            def dense_shard(l, a_shard):
                for nt in range(cfg.nb):
                    pd_ = dpsum.tile([P, HID], F32, tag="dps", name="pd2")
                    for fc in range(cfg.fh):
                        nc.tensor.matmul(
                            out=pd_[:],
                            lhsT=hT[(fc, nt)][:],
                            rhs=wcat[:, w_off[(l, fc)]:w_off[(l, fc)] + HID],
                            start=(fc == 0), stop=(fc == cfg.fh - 1))
                    asb = apool.tile([P, HID], BF16, tag="asb", name="asb2")
                    nc.scalar.copy(out=asb[:], in_=pd_[:])
                    nc.sync.dma_start(
                        out=a_shard[nt * P:(nt + 1) * P, :], in_=asb[:])

            def aggregate(l, a_full):
                for g in range(cfg.NG):
                    pb = {}
                    for h in range(2):
                        call_off = (g * 2 + h) * cfg.G * T * 8
                        M = mpool.tile([P, cfg.G * T * HID], BF16, tag="M", name="M")
                        src_ap = (a_full[0:cfg.half, :] if h == 0
                                  else a_full[cfg.half:cfg.npad, :])
                        CT = 8          # tiles per gather call (<=1024 idx)
                        for k0 in range(0, cfg.G * T, CT):
                            k1 = min(k0 + CT, cfg.G * T)
                            nt_ = k1 - k0
                            nc.gpsimd.dma_gather(
                                out_ap=M[:, k0 * HID:k1 * HID].rearrange(
                                    "p (t e) -> p t e", e=HID),
                                in_ap=src_ap,
                                idxs_ap=gidx[:, call_off + k0 * 8:
                                             call_off + k1 * 8],
                                num_idxs=nt_ * P,
                                num_idxs_reg=nt_ * P,
                                elem_size=HID,
                            )
                        S = spool.tile([P, cfg.G * T * P], BF16, tag="S", name="S")
                        for bg in range(cfg.G):
                            slot0 = ((g * 2 + h) * cfg.G + bg) * T
                            s3 = S[:, bg * T * P:(bg + 1) * T * P].rearrange(
                                "p (t e) -> p t e", e=P)
                            iob = iota[:].rearrange(
                                "p (o e) -> p o e", o=1).broadcast_to([P, T, P])
                            nc.vector.tensor_tensor(
                                out=s3, in0=iob,
                                in1=dstc[:, slot0:slot0 + T].to_broadcast([P, T, P]),
                                op=mybir.AluOpType.is_equal)
                            nc.vector.tensor_tensor(
                                out=s3, in0=s3,
                                in1=normc[:, slot0:slot0 + T].to_broadcast([P, T, P]),
                                op=mybir.AluOpType.mult)
                        for bg in range(cfg.G):
                            if h == 0:
                                pb[bg] = ppool.tile([P, HID], F32, tag="pb", name=f"pb")
                            for t in range(T):
                                tl = bg * T + t
                                for fh in range(cfg.fh):
                                    nc.tensor.matmul(
                                        out=pb[bg][:, fh * P:(fh + 1) * P],
                                        lhsT=M[:, tl * HID + fh * P:
                                               tl * HID + (fh + 1) * P],
                                        rhs=S[:, tl * P:(tl + 1) * P],
                                        start=(h == 0 and t == 0 and fh == 0),
                                        stop=(h == 1 and t == T - 1 and fh == 1))
                    # epilogue for the group's blocks
                    for bg in range(cfg.G):
                        nt = g * cfg.G + bg
                        for fh in range(cfg.fh):
                            pslice = pb[bg][:, fh * P:(fh + 1) * P]
                            bcol = bias[:, l * 2 + fh:l * 2 + fh + 1]
                            if l < 3:
                                ht = htpool.tile([P, P], BF16,
                                                 tag=f"hT{fh}_{nt}", name=f"hT{fh}_{nt}")
                                nc.scalar.activation(
                                    out=ht[:], in_=pslice,
                                    func=mybir.ActivationFunctionType.Identity,
                                    bias=bcol, scale=1.0)
                                hT[(fh, nt)] = ht
                            else:
                                acol = prelua[:, fh:fh + 1]
                                neg = opool.tile([P, P], F32, tag="neg", name="neg")
                                nc.vector.tensor_scalar(
                                    out=neg[:], in0=pslice,
                                    scalar1=bcol, scalar2=0.0,
                                    op0=mybir.AluOpType.add,
                                    op1=mybir.AluOpType.min)
                                pos = opool.tile([P, P], F32, tag="pos", name="pos")
                                nc.vector.tensor_scalar(
                                    out=pos[:], in0=pslice,
                                    scalar1=bcol, scalar2=0.0,
                                    op0=mybir.AluOpType.add,
                                    op1=mybir.AluOpType.max)
                                nc.vector.tensor_scalar(
                                    out=neg[:], in0=neg[:],
                                    scalar1=acol, scalar2=None,
                                    op0=mybir.AluOpType.mult)
                                osb = opool.tile([P, P], F32, tag="osb", name="osb")
                                nc.vector.tensor_tensor(
                                    out=osb[:], in0=pos[:], in1=neg[:],
                                    op=mybir.AluOpType.add)
                                base = (fh * cfg.nb + nt) * P
                                nc.sync.dma_start(
                                    out=out_d[base:base + P, :], in_=osb[:])

            # ---- layer 1
            a_full = dram.tile([cfg.npad, HID], BF16, tag="afull", name="afull1")
            dense_full_l1(a_full)
            if debug_dense:
                rows = cfg.fh * cfg.nb * P
                nc.gpsimd.dma_start(out=out_d[:, :],
                                    in_=a_full[0:rows, 0:P])
                nc.compile()
                return nc
            aggregate(0, a_full)
            # ---- layers 2..4
            for l in range(1, n_layers):
                a_shard = dram.tile([cfg.shard, HID], BF16, tag="ashard", name="ashard")
                dense_shard(l, a_shard)
                a_full = dram.tile([cfg.npad, HID], BF16, tag="afull", name="afull")
                nc.gpsimd.collective_compute(
                    "AllGather",
                    mybir.AluOpType.bypass,
                    ins=[a_shard[:].opt()],
                    outs=[a_full[:].opt()],
                    replica_groups=[list(range(cfg.ncores))],
                )
                aggregate(l, a_full)

            if n_layers < 4:
                # debug: dump hT tiles (post-bias h of layer n_layers) to out
                for nt in range(cfg.nb):
                    for fh in range(cfg.fh):
                        osb = opool.tile([P, P], F32, tag="osb", name="osbd")
                        nc.vector.tensor_copy(out=osb[:], in_=hT[(fh, nt)][:])
                        base = (fh * cfg.nb + nt) * P
                        nc.sync.dma_start(out=out_d[base:base + P, :], in_=osb[:])

    nc.compile()
    return nc


# ----------------------------------------------------------------- execution

def _make_in_maps(cfg, prep, x, Ws, bs, prelu_a):
    xts = _pack_xts(cfg, np.asarray(x, np.float32), prep["gslot"])
    wcat = _pack_wcat(cfg, Ws)
    biasp = _pack_bias(cfg, bs)
    prelup = np.zeros((P, 2), np.float32)
    prelup[:, 0] = prelu_a[:P]
    prelup[:, 1] = prelu_a[P:]
    iota = _iota_np()
    maps = []
    for c in range(cfg.ncores):
        maps.append({
            "gidx": prep["gidx"][c],
            "dstc": prep["dstc"][c],
            "normc": prep["normc"][c],
            "iota": iota,
            "wcat": wcat,
            "bias": biasp,
            "prelua": prelup,
            "xts": xts,
        })
    return maps


def _assemble_out(cfg, results, gslot):
    """results: list of per-core {'out': [fh*nb*128, 128]} -> y [n_nodes, hid]."""
    cores = []
    for c in range(cfg.ncores):
        o = results[c]["out"].reshape(cfg.fh, cfg.nb, P, P)
        # o[fh, nt, p, cpos] = h[f = fh*128+p, local slot = nt*128+cpos]
        oT = o.transpose(0, 2, 1, 3).reshape(cfg.hid, cfg.shard)
        cores.append(oT)
    yperm = np.concatenate(cores, axis=1).T        # [npad, hid]
    return np.ascontiguousarray(yperm[gslot]).astype(np.float32)


def run(cfg, x, edge_index, edge_weight, W1, b1, W2, b2, W3, b3, W4, b4,
        prelu_a, return_nc=False):
    prep = _preprocess(cfg, edge_index, edge_weight)
    nc = _build(cfg, prep["T"])
    in_maps = _make_in_maps(cfg, prep, x,
                            [np.asarray(W1, np.float32), np.asarray(W2, np.float32),
                             np.asarray(W3, np.float32), np.asarray(W4, np.float32)],
                            [np.asarray(b1, np.float32), np.asarray(b2, np.float32),
                             np.asarray(b3, np.float32), np.asarray(b4, np.float32)],
                            np.asarray(prelu_a, np.float32))
    res = run_bass_kernel_spmd(nc, in_maps, core_ids=list(range(cfg.ncores)))
    y = _assemble_out(cfg, res.results, prep["gslot"])
    if return_nc:
        return y, nc, in_maps
    return y


def kernel(x, edge_index, edge_weight, W1, b1, W2, b2, W3, b3, W4, b4, prelu_a):
    return run(CFG, x, edge_index, edge_weight,
               W1, b1, W2, b2, W3, b3, W4, b4, prelu_a)


# revision 9
# speedup vs baseline: 1.4588x; 1.4588x over previous
"""4-layer GCN encoder on 8 Trainium2 NeuronCores.

Strategy (graph/data parallel, dst-node sharding):
  - Nodes are permuted into 8*NB blocks of 128 (balanced by in-degree) and
    sharded across 8 cores by destination.
  - Layer 1 dense projection x@W1 is computed fully on every core (cheaper
    than an AllGather of the result); layers 2-4 compute only the local node
    shard and AllGather the projected features.
  - Aggregation: per-edge gather of source rows via the SWDGE dma_gather
    instruction (int16 indices -> source-half split), then scatter-add via
    TensorEngine matmuls against on-chip-generated one-hot matrices S with
    the per-edge GCN norm as values (PSUM accumulation per dst block).
  - All matmul operands bf16, accumulation fp32.
"""

import math
import numpy as np
import ml_dtypes

import concourse.bacc as bacc
import concourse.mybir as mybir
import concourse.tile as tile
from concourse.bass_utils import run_bass_kernel_spmd

P = 128
BF16 = mybir.dt.bfloat16
F32 = mybir.dt.float32
I16 = mybir.dt.int16


class Cfg:
    def __init__(self, n_nodes=50000, n_edges=800000, in_ch=512, hid=256,
                 ncores=8, nb=49, G=7):
        self.n_nodes = n_nodes
        self.n_edges = n_edges
        self.in_ch = in_ch
        self.hid = hid
        self.ncores = ncores
        self.nb = nb                      # dst blocks of 128 per core
        self.G = G                        # blocks per gather group
        assert nb % G == 0
        self.NG = nb // G                 # groups per core
        self.shard = nb * P               # nodes per core (padded)
        self.npad = ncores * self.shard   # padded total nodes
        assert self.npad >= n_nodes
        self.half = self.npad // 2        # src-half boundary for int16 idx
        assert self.half % P == 0 and self.half < 32768
        self.fc_in = in_ch // P           # K chunks for layer 1
        self.fh = hid // P                # feature halves (2)
        assert self.fh == 2
        self.slab = 8                     # n-tiles per L1 x slab


CFG = Cfg()


# ----------------------------------------------------------------- host prep

def _preprocess(cfg, edge_index, edge_weight):
    """Numpy preprocessing: norms, balanced node permutation, per-core
    padded edge structures. Returns dict."""
    N = cfg.n_nodes
    src0 = np.asarray(edge_index[0], dtype=np.int64)
    dst0 = np.asarray(edge_index[1], dtype=np.int64)
    ew0 = np.asarray(edge_weight, dtype=np.float32)
    # self loops (PyG gcn_norm, fill=1)
    loops = np.arange(N, dtype=np.int64)
    src = np.concatenate([src0, loops])
    dst = np.concatenate([dst0, loops])
    ew = np.concatenate([ew0, np.ones(N, np.float32)])
    deg = np.bincount(dst, weights=ew.astype(np.float64), minlength=N)
    deg = deg.astype(np.float32)
    dis = np.where(deg > 0, 1.0 / np.sqrt(np.where(deg > 0, deg, 1.0)), 0.0)
    dis = dis.astype(np.float32)
    norm = dis[src] * ew * dis[dst]

    # balanced block assignment: round-robin of degree-sorted nodes
    NBT = cfg.ncores * cfg.nb
    degc = np.bincount(dst, minlength=N)          # in-edge counts per node
    order = np.argsort(-degc, kind="stable")
    blk_of_rank = np.arange(N) % NBT
    pos_of_rank = np.arange(N) // NBT
    assert pos_of_rank.max() < P, "block capacity exceeded"
    gslot = np.empty(N, dtype=np.int64)
    gslot[order] = blk_of_rank * P + pos_of_rank

    ps = gslot[src]
    pd = gslot[dst]

    # per (core, block, half) edge counts to find uniform tile count T
    eb = pd // P                                  # global dst block per edge
    ehalf = (ps >= cfg.half).astype(np.int64)
    key = eb * 2 + ehalf
    cnt = np.bincount(key, minlength=NBT * 2)
    T = max(1, int(np.ceil(cnt.max() / P)))

    nslots = cfg.nb * 2 * T                       # tiles per core
    cap = nslots * P
    gidx16 = np.zeros((cfg.ncores, 16, cap // 16), dtype=np.int16)
    dstc = np.zeros((cfg.ncores, P, nslots), dtype=np.float32)
    normc = np.zeros((cfg.ncores, P, nslots), dtype=np.float32)

    # global ordering of edges: core -> (group, half, block-in-group, tile)
    core_e = eb // cfg.nb
    b_in_core = eb % cfg.nb
    g = b_in_core // cfg.G
    bg = b_in_core % cfg.G
    # slot (tile) base for each edge's (b, h) bucket
    srt = np.lexsort((ps, ehalf, eb))             # sort edges by (block, half, src)
    # rank within bucket
    key_s = key[srt]
    # compute rank-in-bucket via cumcount
    uniq, inv, counts = np.unique(key_s, return_inverse=True, return_counts=True)
    starts = np.zeros_like(counts)
    starts[1:] = np.cumsum(counts)[:-1]
    rank_in_bucket = np.arange(len(srt)) - starts[inv]

    es = srt                                      # edge order
    t_idx = rank_in_bucket // P                   # tile within bucket
    j_idx = rank_in_bucket % P                    # lane within tile
    assert t_idx.max() < T
    sg = g[es]
    sh = ehalf[es]
    sbg = bg[es]
    s_slot = ((sg * 2 + sh) * cfg.G + sbg) * T + t_idx
    q = s_slot * P + j_idx                        # position within core arrays
    score = core_e[es]
    idxval = np.where(sh == 1, ps[es] - cfg.half, ps[es]).astype(np.int16)
    dlocal = (pd[es] % P).astype(np.float32)
    nval = norm[es]

    for c in range(cfg.ncores):
        m = score == c
        qc = q[m]
        gidx16[c, qc % 16, qc // 16] = idxval[m]
        dstc[c, qc % P, qc // P] = dlocal[m]
        normc[c, qc % P, qc // P] = nval[m]

    gidx = np.tile(gidx16, (1, 8, 1))             # replicate to 128 partitions
    inv_gslot = gslot                             # y[v] = yperm[gslot[v]]
    return dict(T=T, nslots=nslots, gidx=gidx,
                dstc=dstc.astype(ml_dtypes.bfloat16),
                normc=normc.astype(ml_dtypes.bfloat16),
                gslot=inv_gslot)


def _pack_xts(cfg, x, gslot):
    """Host: permuted, transposed, slab-tiled x for layer-1 lhsT streaming.
    Layout [fc, s, p, t*128+c] = x_perm[(s*8+t)*128+c, fc*128+p]."""
    xpad = np.zeros((cfg.npad, cfg.in_ch), dtype=np.float32)
    xpad[gslot] = x
    nslab = cfg.npad // (cfg.slab * P)
    a = xpad.T.reshape(cfg.fc_in, P, nslab, cfg.slab, P)
    a = a.transpose(0, 2, 1, 3, 4).reshape(cfg.fc_in, nslab, P, cfg.slab * P)
    return np.ascontiguousarray(a.astype(ml_dtypes.bfloat16)).reshape(
        cfg.fc_in * nslab * P, cfg.slab * P)


def _pack_wcat(cfg, Ws):
    """[128, (fc_in + 3*fh)*hid] bf16 : W1 chunks then W2..W4 chunks."""
    cols = []
    for Wl in Ws:
        k = Wl.shape[0]
        for fc in range(k // P):
            cols.append(Wl[fc * P:(fc + 1) * P, :])
    return np.concatenate(cols, axis=1).astype(ml_dtypes.bfloat16)


def _pack_bias(cfg, bs):
    out = np.zeros((P, 2 * len(bs)), dtype=np.float32)
    for l, b in enumerate(bs):
        for fh in range(cfg.fh):
            out[:, l * 2 + fh] = b[fh * P:(fh + 1) * P]
    return out


def _iota_np():
    return np.tile(np.arange(P, dtype=np.float32)[None, :], (P, 1)).astype(
        ml_dtypes.bfloat16)


# ----------------------------------------------------------------- builder

def _build(cfg, T, n_layers=4, debug_dense=False):
    nslots = cfg.nb * 2 * T
    HID = cfg.hid
    nc = bacc.Bacc("TRN2", target_bir_lowering=False, debug=False,
                   num_devices=cfg.ncores, num_swdge_queues=4)
    qctr = [0]

    gidx_d = nc.dram_tensor("gidx", [P, nslots * 8], I16, kind="ExternalInput")
    dstc_d = nc.dram_tensor("dstc", [P, nslots], BF16, kind="ExternalInput")
    normc_d = nc.dram_tensor("normc", [P, nslots], BF16, kind="ExternalInput")
    iota_d = nc.dram_tensor("iota", [P, P], BF16, kind="ExternalInput")
    wcat_cols = (cfg.fc_in + 3 * cfg.fh) * HID
    wcat_d = nc.dram_tensor("wcat", [P, wcat_cols], BF16, kind="ExternalInput")
    bias_d = nc.dram_tensor("bias", [P, 8], F32, kind="ExternalInput")
    prelu_d = nc.dram_tensor("prelua", [P, 2], F32, kind="ExternalInput")
    nslab = cfg.npad // (cfg.slab * P)
    xts_d = nc.dram_tensor("xts", [cfg.fc_in * nslab * P, cfg.slab * P], BF16,
                           kind="ExternalInput")
    out_d = nc.dram_tensor("out", [cfg.fh * cfg.nb * P, P], F32,
                           kind="ExternalOutput")

    w_off = {}
    off = 0
    for l in range(4):
        k = cfg.fc_in if l == 0 else cfg.fh
        for fc in range(k):
            w_off[(l, fc)] = off
            off += HID

    with tile.TileContext(nc) as tc:
        with (
            tc.tile_pool(name="res", bufs=1) as res,
            tc.tile_pool(name="mpool", bufs=2) as mpool,
            tc.tile_pool(name="spool", bufs=2) as spool,
            tc.tile_pool(name="xpool", bufs=2) as xpool,
            tc.tile_pool(name="apool", bufs=4) as apool,
            tc.tile_pool(name="htpool", bufs=1) as htpool,
            tc.tile_pool(name="opool", bufs=4) as opool,
            tc.tile_pool(name="ppool", bufs=cfg.G, space="PSUM") as ppool,
            tc.tile_pool(name="dpsum", bufs=1, space="PSUM") as dpsum,
            tc.tile_pool(name="dram", bufs=2, space="DRAM") as dram,
        ):
            # ---- resident loads
            gidx = res.tile([P, nslots * 8], I16)
            nc.sync.dma_start(out=gidx[:], in_=gidx_d[:])
            dstc = res.tile([P, nslots], BF16)
            nc.sync.dma_start(out=dstc[:], in_=dstc_d[:])
            normc = res.tile([P, nslots], BF16)
            nc.sync.dma_start(out=normc[:], in_=normc_d[:])
            iota = res.tile([P, P], BF16)
            nc.sync.dma_start(out=iota[:], in_=iota_d[:])
            wcat = res.tile([P, wcat_cols], BF16)
            nc.sync.dma_start(out=wcat[:], in_=wcat_d[:])
            bias = res.tile([P, 8], F32)
            nc.sync.dma_start(out=bias[:], in_=bias_d[:])
            prelua = res.tile([P, 2], F32)
            nc.sync.dma_start(out=prelua[:], in_=prelu_d[:])

            hT = {}

            def dense_full_l1(a_full):
                for s in range(nslab):
                    xsl = [xpool.tile([P, cfg.slab * P], BF16, tag=f"x{fc}", name=f"xsl{fc}")
                           for fc in range(cfg.fc_in)]
                    for fc in range(cfg.fc_in):
                        base = (fc * nslab + s) * P
                        nc.sync.dma_start(out=xsl[fc][:],
                                          in_=xts_d[base:base + P, :])
                    for t in range(cfg.slab):
                        nt = s * cfg.slab + t
                        pd_ = dpsum.tile([P, HID], F32, tag="dps", name="pd1")
                        for fc in range(cfg.fc_in):
                            nc.tensor.matmul(
                                out=pd_[:],
                                lhsT=xsl[fc][:, t * P:(t + 1) * P],
                                rhs=wcat[:, w_off[(0, fc)]:w_off[(0, fc)] + HID],
                                start=(fc == 0), stop=(fc == cfg.fc_in - 1))
                        asb = apool.tile([P, HID], BF16, tag="asb", name="asb1")
                        nc.scalar.copy(out=asb[:], in_=pd_[:])
                        nc.sync.dma_start(
                            out=a_full[nt * P:(nt + 1) * P, :], in_=asb[:])

            def dense_shard(l, a_shard):
                for nt in range(cfg.nb):
                    pd_ = dpsum.tile([P, HID], F32, tag="dps", name="pd2")
                    for fc in range(cfg.fh):
                        nc.tensor.matmul(
                            out=pd_[:],
                            lhsT=hT[(fc, nt)][:],
                            rhs=wcat[:, w_off[(l, fc)]:w_off[(l, fc)] + HID],
                            start=(fc == 0), stop=(fc == cfg.fh - 1))
                    asb = apool.tile([P, HID], BF16, tag="asb", name="asb2")
                    nc.scalar.copy(out=asb[:], in_=pd_[:])
                    nc.sync.dma_start(
                        out=a_shard[nt * P:(nt + 1) * P, :], in_=asb[:])

            def aggregate(l, a_full):
                for g in range(cfg.NG):
                    pb = {}
                    for h in range(2):
                        call_off = (g * 2 + h) * cfg.G * T * 8
                        M = mpool.tile([P, cfg.G * T * HID], BF16, tag="M", name="M")
                        src_ap = (a_full[0:cfg.half, :] if h == 0
                                  else a_full[cfg.half:cfg.npad, :])
                        CT = 8          # tiles per gather call (<=1024 idx)
                        for k0 in range(0, cfg.G * T, CT):
                            k1 = min(k0 + CT, cfg.G * T)
                            nt_ = k1 - k0
                            nc.gpsimd.dma_gather(
                                out_ap=M[:, k0 * HID:k1 * HID].rearrange(
                                    "p (t e) -> p t e", e=HID),
                                in_ap=src_ap,
                                idxs_ap=gidx[:, call_off + k0 * 8:
                                             call_off + k1 * 8],
                                num_idxs=nt_ * P,
                                num_idxs_reg=nt_ * P,
                                elem_size=HID,
                                queue_num=qctr[0] % 4,
                            )
                            qctr[0] += 1
                        S = spool.tile([P, cfg.G * T * P], BF16, tag="S", name="S")
                        for bg in range(cfg.G):
                            slot0 = ((g * 2 + h) * cfg.G + bg) * T
                            s3 = S[:, bg * T * P:(bg + 1) * T * P].rearrange(
                                "p (t e) -> p t e", e=P)
                            iob = iota[:].rearrange(
                                "p (o e) -> p o e", o=1).broadcast_to([P, T, P])
                            nc.vector.tensor_tensor(
                                out=s3, in0=iob,
                                in1=dstc[:, slot0:slot0 + T].to_broadcast([P, T, P]),
                                op=mybir.AluOpType.is_equal)
                            nc.vector.tensor_tensor(
                                out=s3, in0=s3,
                                in1=normc[:, slot0:slot0 + T].to_broadcast([P, T, P]),
                                op=mybir.AluOpType.mult)
                        for bg in range(cfg.G):
                            if h == 0:
                                pb[bg] = ppool.tile([P, HID], F32, tag="pb", name=f"pb")
                            for t in range(T):
                                tl = bg * T + t
                                for fh in range(cfg.fh):
                                    nc.tensor.matmul(
                                        out=pb[bg][:, fh * P:(fh + 1) * P],
                                        lhsT=M[:, tl * HID + fh * P:
                                               tl * HID + (fh + 1) * P],
                                        rhs=S[:, tl * P:(tl + 1) * P],
                                        start=(h == 0 and t == 0 and fh == 0),
                                        stop=(h == 1 and t == T - 1 and fh == 1))
                    # epilogue for the group's blocks
                    for bg in range(cfg.G):
                        nt = g * cfg.G + bg
                        for fh in range(cfg.fh):
                            pslice = pb[bg][:, fh * P:(fh + 1) * P]
                            bcol = bias[:, l * 2 + fh:l * 2 + fh + 1]
                            if l < 3:
                                ht = htpool.tile([P, P], BF16,
                                                 tag=f"hT{fh}_{nt}", name=f"hT{fh}_{nt}")
                                nc.scalar.activation(
                                    out=ht[:], in_=pslice,
                                    func=mybir.ActivationFunctionType.Identity,
                                    bias=bcol, scale=1.0)
                                hT[(fh, nt)] = ht
                            else:
                                acol = prelua[:, fh:fh + 1]
                                neg = opool.tile([P, P], F32, tag="neg", name="neg")
                                nc.vector.tensor_scalar(
                                    out=neg[:], in0=pslice,
                                    scalar1=bcol, scalar2=0.0,
                                    op0=mybir.AluOpType.add,
                                    op1=mybir.AluOpType.min)
                                pos = opool.tile([P, P], F32, tag="pos", name="pos")
                                nc.vector.tensor_scalar(
                                    out=pos[:], in0=pslice,
                                    scalar1=bcol, scalar2=0.0,
                                    op0=mybir.AluOpType.add,
                                    op1=mybir.AluOpType.max)
                                nc.vector.tensor_scalar(
                                    out=neg[:], in0=neg[:],
                                    scalar1=acol, scalar2=None,
                                    op0=mybir.AluOpType.mult)
                                osb = opool.tile([P, P], F32, tag="osb", name="osb")
                                nc.vector.tensor_tensor(
                                    out=osb[:], in0=pos[:], in1=neg[:],
                                    op=mybir.AluOpType.add)
                                base = (fh * cfg.nb + nt) * P
                                nc.sync.dma_start(
                                    out=out_d[base:base + P, :], in_=osb[:])

            # ---- layer 1
            a_full = dram.tile([cfg.npad, HID], BF16, tag="afull", name="afull1")
            dense_full_l1(a_full)
            if debug_dense:
                rows = cfg.fh * cfg.nb * P
                nc.gpsimd.dma_start(out=out_d[:, :],
                                    in_=a_full[0:rows, 0:P])
                nc.compile()
                return nc
            aggregate(0, a_full)
            # ---- layers 2..4
            for l in range(1, n_layers):
                a_shard = dram.tile([cfg.shard, HID], BF16, tag="ashard", name="ashard")
                dense_shard(l, a_shard)
                a_full = dram.tile([cfg.npad, HID], BF16, tag="afull", name="afull")
                nc.gpsimd.collective_compute(
                    "AllGather",
                    mybir.AluOpType.bypass,
                    ins=[a_shard[:].opt()],
                    outs=[a_full[:].opt()],
                    replica_groups=[list(range(cfg.ncores))],
                )
                aggregate(l, a_full)

            if n_layers < 4:
                # debug: dump hT tiles (post-bias h of layer n_layers) to out
                for nt in range(cfg.nb):
                    for fh in range(cfg.fh):
                        osb = opool.tile([P, P], F32, tag="osb", name="osbd")
                        nc.vector.tensor_copy(out=osb[:], in_=hT[(fh, nt)][:])
                        base = (fh * cfg.nb + nt) * P
                        nc.sync.dma_start(out=out_d[base:base + P, :], in_=osb[:])

    nc.compile()
    return nc


# ----------------------------------------------------------------- execution

def _make_in_maps(cfg, prep, x, Ws, bs, prelu_a):
    xts = _pack_xts(cfg, np.asarray(x, np.float32), prep["gslot"])
    wcat = _pack_wcat(cfg, Ws)
    biasp = _pack_bias(cfg, bs)
    prelup = np.zeros((P, 2), np.float32)
    prelup[:, 0] = prelu_a[:P]
    prelup[:, 1] = prelu_a[P:]
    iota = _iota_np()
    maps = []
    for c in range(cfg.ncores):
        maps.append({
            "gidx": prep["gidx"][c],
            "dstc": prep["dstc"][c],
            "normc": prep["normc"][c],
            "iota": iota,
            "wcat": wcat,
            "bias": biasp,
            "prelua": prelup,
            "xts": xts,
        })
    return maps


def _assemble_out(cfg, results, gslot):
    """results: list of per-core {'out': [fh*nb*128, 128]} -> y [n_nodes, hid]."""
    cores = []
    for c in range(cfg.ncores):
        o = results[c]["out"].reshape(cfg.fh, cfg.nb, P, P)
        # o[fh, nt, p, cpos] = h[f = fh*128+p, local slot = nt*128+cpos]
        oT = o.transpose(0, 2, 1, 3).reshape(cfg.hid, cfg.shard)
        cores.append(oT)
    yperm = np.concatenate(cores, axis=1).T        # [npad, hid]
    return np.ascontiguousarray(yperm[gslot]).astype(np.float32)


def run(cfg, x, edge_index, edge_weight, W1, b1, W2, b2, W3, b3, W4, b4,
        prelu_a, return_nc=False):
    prep = _preprocess(cfg, edge_index, edge_weight)
    nc = _build(cfg, prep["T"])
    in_maps = _make_in_maps(cfg, prep, x,
                            [np.asarray(W1, np.float32), np.asarray(W2, np.float32),
                             np.asarray(W3, np.float32), np.asarray(W4, np.float32)],
                            [np.asarray(b1, np.float32), np.asarray(b2, np.float32),
                             np.asarray(b3, np.float32), np.asarray(b4, np.float32)],
                            np.asarray(prelu_a, np.float32))
    res = run_bass_kernel_spmd(nc, in_maps, core_ids=list(range(cfg.ncores)))
    y = _assemble_out(cfg, res.results, prep["gslot"])
    if return_nc:
        return y, nc, in_maps
    return y


def kernel(x, edge_index, edge_weight, W1, b1, W2, b2, W3, b3, W4, b4, prelu_a):
    return run(CFG, x, edge_index, edge_weight,
               W1, b1, W2, b2, W3, b3, W4, b4, prelu_a)
